# revision 10
# baseline (speedup 1.0000x reference)
"""Causal GQA self-attention block (B=2, S=2048, HID=768, 12 q heads, 4 kv heads)
distributed over 8 Trainium2 NeuronCores.

Sharding: one core per (batch, kv-head-group) pair -> 8 perfectly balanced
shards.  Each core projects q/k/v for its group (3 query heads + 1 kv head),
runs causal attention for its 3 heads over the full sequence, and applies its
row-shard of the output projection (rows belonging to its 3 heads' features),
producing a partial [S, HID] output.  The host sums the 4 partials per batch
(the row-sharded matmul unshard step) and stacks the 2 batches.

On-device layout (per core):
  xT   [768, 2048]  hidden-major activations (host passes x[b].T)
  qT_h [64, 2048] x3, kT [64, 2048], vT [64, 2048] (feature-major)
  scores are computed transposed: st[sk, sq] = kT.T-tile @ qT -> softmax along
  the partition axis is realized with an extra all-ones row appended to V
  (denominators fall out of the attention*V matmul for free), causal masking
  by elementwise 0/1 masks on the exp'd tiles, and a K=1 broadcast matmul to
  divide by the denominators at the end.
All matmuls run in float32r (full PE rate for moving dim >= 256).
"""

import numpy as np

import concourse.bass as bass
import concourse.mybir as mybir
import concourse.tile as tile
from concourse.bass_utils import run_bass_kernel_spmd

F32 = mybir.dt.float32
F32R = mybir.dt.float32r
AF = mybir.ActivationFunctionType

B, S, HID = 2, 2048, 768
KV, G, D = 4, 3, 64
SCALE = D**-0.5
N_CORES = 8

ST = S // 128  # 16 sequence tiles of 128
SC = S // 512  # 4 sequence chunks of 512
HC = HID // 128  # 6 hidden chunks of 128
NG = G + 2  # projection feature groups: q0 q1 q2 k v


def _split_excess_waits(nc, max_waits=1):
    """This walrus build rejects >1 sync wait per instruction; move extras
    onto engine-matched NoOps placed right before the instruction."""
    for f in nc.m.functions:
        for bb in f.blocks:
            insts = list(bb.instructions)
            out = []
            changed = False
            for inst in insts:
                si = getattr(inst, "sync_info", None)
                waits = list(si.on_wait) if si is not None else []
                if len(waits) > max_waits:
                    for w in waits[max_waits:]:
                        out.append(
                            mybir.InstNoOp(
                                name=nc.get_next_instruction_name(),
                                sync_info=mybir.SyncInfo(on_wait=[w], on_update=[]),
                                bass_nofuse=True,
                                engine=inst.engine,
                            )
                        )
                    si.on_wait = waits[:max_waits]
                    changed = True
                out.append(inst)
            if changed:
                try:
                    bb.instructions = out
                except Exception:
                    cur = bb.instructions
                    cur.clear()
                    cur.extend(out)


def _causal_masks():
    # st tile (sk-tile p within an sq-chunk, rows i=sk%128, cols q=sq%512):
    # valid iff 128*p + i <= q.
    m = np.zeros((128, 4, 512), np.float32)
    q = np.arange(512)[None, :]
    i = np.arange(128)[:, None]
    for p in range(4):
        m[:, p, :] = (128 * p + i <= q).astype(np.float32)
    return m


def _build_program():
    nc = bass.Bass()
    xT = nc.dram_tensor("xT", [HID, S], F32, kind="ExternalInput")
    Wall = nc.dram_tensor("Wall", [HID, NG * D], F32, kind="ExternalInput")
    bqkv = nc.dram_tensor("bqkv", [NG * D, 1], F32, kind="ExternalInput")
    Wo = nc.dram_tensor("Wo", [G * D, HID], F32, kind="ExternalInput")
    bo4 = nc.dram_tensor("bo4", [HID], F32, kind="ExternalInput")
    out = nc.dram_tensor("out", [S, HID], F32, kind="ExternalOutput")

    masks_d = nc.inline_tensor(_causal_masks(), name="cmasks")
    eye_d = nc.inline_tensor(np.eye(D, dtype=np.float32), name="eye64")
    ones_d = nc.inline_tensor(np.ones((1, D), np.float32), name="ones64")
    onescol_d = nc.inline_tensor(np.ones((128, ST, 1), np.float32), name="onescol")

    bo4_bcast = bass.AP(tensor=bo4.ap().tensor, offset=0, ap=[[0, 128], [1, HID]])

    with tile.TileContext(nc) as tc:
        with (
            tc.tile_pool(name="persist", bufs=1) as pp,
            tc.tile_pool(name="exp", bufs=6) as ep,
            tc.tile_pool(name="stage", bufs=3) as sp,
            tc.tile_pool(name="ps_mm", bufs=2, space="PSUM") as ps_mm,
            tc.tile_pool(name="ps_acc", bufs=2, space="PSUM") as ps_acc,
            tc.tile_pool(name="ps_out", bufs=2, space="PSUM") as ps_out,
        ):
            # ---- resident inputs -------------------------------------------------
            xt = pp.tile([128, HC, S], F32R, tag="xT")
            for c in range(HC):
                nc.sync.dma_start(
                    out=xt[:, c, :], in_=xT[c * 128 : (c + 1) * 128, :].bitcast(F32R)
                )
            wt = pp.tile([128, HC, NG * D], F32R, tag="Wall")
            for c in range(HC):
                nc.sync.dma_start(
                    out=wt[:, c, :],
                    in_=Wall[c * 128 : (c + 1) * 128, :].bitcast(F32R),
                )
            bt = pp.tile([D, NG], F32, tag="bqkv")
            for g in range(NG):
                nc.sync.dma_start(out=bt[:, g : g + 1], in_=bqkv[g * D : (g + 1) * D, :])
            wo = pp.tile([D, G, HID], F32R, tag="Wo")
            for h in range(G):
                nc.sync.dma_start(
                    out=wo[:, h, :], in_=Wo[h * D : (h + 1) * D, :].bitcast(F32R)
                )
            bo_t = pp.tile([128, HID], F32, tag="bo4")
            nc.sync.dma_start(out=bo_t, in_=bo4_bcast)
            mk = pp.tile([128, 4, 512], F32R, tag="masks")
            nc.sync.dma_start(out=mk, in_=masks_d.ap().bitcast(F32R))
            eye = pp.tile([D, D], F32R, tag="eye")
            nc.sync.dma_start(out=eye, in_=eye_d.ap().bitcast(F32R))
            # all-ones row living at partition D (=64) so the K=1 broadcast
            # matmul's operands share a base partition
            ones_t = pp.tile([D + 1, D], F32R, tag="ones")
            nc.sync.dma_start(out=ones_t[D : D + 1, :], in_=ones_d.ap().bitcast(F32R))

            # ---- q/k/v projections (feature-major) -------------------------------
            # gT[g] [64, S] = (x @ Wall[:, g*64:(g+1)*64]).T + b[g]
            gts = [pp.tile([D, S], F32R, tag=f"gT{g}", name=f"gT{g}") for g in range(NG)]
            for g in range(NG):
                for j in range(SC):
                    acc = ps_acc.tile([D, 512], F32, tag="acc")
                    for c in range(HC):
                        nc.tensor.matmul(
                            acc,
                            wt[:, c, g * D : (g + 1) * D],
                            xt[:, c, j * 512 : (j + 1) * 512],
                            start=(c == 0),
                            stop=(c == HC - 1),
                        )
                    nc.vector.tensor_scalar_add(
                        gts[g][:, j * 512 : (j + 1) * 512], acc, bt[:, g : g + 1]
                    )
            qts, kt, vt = gts[:G], gts[G], gts[G + 1]

            # ---- v' = [v | 1] in sequence-major layout ---------------------------
            vv = pp.tile([128, ST, D + 1], F32R, tag="vv")
            for t in range(ST):
                tp = ps_acc.tile([128, D], F32R, tag="acc")
                nc.tensor.transpose(tp, vt[:, t * 128 : (t + 1) * 128], eye)
                nc.vector.tensor_copy(vv[:, t, 0:D], tp)
            nc.sync.dma_start(
                out=vv[:, :, D : D + 1],
                in_=onescol_d.ap().bitcast(F32R),
            )

            # ---- attention per head ---------------------------------------------
            ots = [pp.tile([D + 1, S], F32R, tag=f"oT{h}", name=f"oT{h}") for h in range(G)]
            for h in range(G):
                qt = qts[h]
                for j in range(SC):
                    n_i = 4 * (j + 1)
                    oacc = ps_acc.tile([D + 1, 512], F32, tag="acc")
                    pend = None  # software-pipeline the AV matmul one step back
                    for i in range(n_i):
                        st = ps_mm.tile([128, 512], F32, tag="mm")
                        nc.tensor.matmul(
                            st,
                            kt[:, i * 128 : (i + 1) * 128],
                            qt[:, j * 512 : (j + 1) * 512],
                            start=True,
                            stop=True,
                        )
                        e = ep.tile([128, 512], F32R, tag="e")
                        nc.scalar.activation(out=e, in_=st, func=AF.Exp, scale=SCALE)
                        if i // 4 == j:
                            nc.vector.tensor_mul(e, e, mk[:, i % 4, :])
                        if pend is not None:
                            nc.tensor.matmul(
                                oacc,
                                vv[:, pend[1], :],
                                pend[0],
                                start=(pend[1] == 0),
                                stop=False,
                            )
                        pend = (e, i)
                    nc.tensor.matmul(
                        oacc,
                        vv[:, pend[1], :],
                        pend[0],
                        start=(pend[1] == 0),
                        stop=True,
                    )
                    nc.scalar.activation(
                        out=ots[h][:, j * 512 : (j + 1) * 512], in_=oacc, func=AF.Copy
                    )
                # normalize: o[d, s] *= 1/denom[s]
                with nc.allow_low_precision(reason="f32r denominators"):
                    nc.vector.reciprocal(
                        out=ots[h][D : D + 1, :], in_=ots[h][D : D + 1, :]
                    )
                for j in range(SC):
                    bc = ps_acc.tile([D, 512], F32, tag="acc")
                    nc.tensor.matmul(
                        bc,
                        ones_t[D : D + 1, :],
                        ots[h][D : D + 1, j * 512 : (j + 1) * 512],
                        start=True,
                        stop=True,
                    )
                    nc.vector.tensor_mul(
                        ots[h][0:D, j * 512 : (j + 1) * 512],
                        ots[h][0:D, j * 512 : (j + 1) * 512],
                        bc.bitcast(F32R),
                    )

            # ---- output projection (row shard) + bias/4 --------------------------
            for t in range(ST):
                po = ps_out.tile([128, HID], F32, tag="out")
                for h in range(G):
                    lhs = ots[h][0:D, t * 128 : (t + 1) * 128]
                    nc.tensor.matmul(
                        po[:, 0:512],
                        lhs,
                        wo[:, h, 0:512],
                        start=(h == 0),
                        stop=(h == G - 1),
                    )
                    nc.tensor.matmul(
                        po[:, 512:HID],
                        lhs,
                        wo[:, h, 512:HID],
                        start=(h == 0),
                        stop=(h == G - 1),
                    )
                stg = sp.tile([128, HID], F32, tag="stg")
                nc.vector.tensor_add(stg, po, bo_t)
                nc.sync.dma_start(out=out[t * 128 : (t + 1) * 128, :], in_=stg)

    _split_excess_waits(nc)
    return nc


_PROGRAM = None


def _program():
    global _PROGRAM
    if _PROGRAM is None:
        _PROGRAM = _build_program()
    return _PROGRAM


def kernel(x, Wq, bq, Wk, bk, Wv, bv, Wo, bo, **_):
    x = np.asarray(x, np.float32)
    in_maps = []
    for core in range(N_CORES):
        b, kv = divmod(core, KV)
        qs = slice(kv * G * D, (kv + 1) * G * D)
        ks = slice(kv * D, (kv + 1) * D)
        in_maps.append(
            {
                "xT": np.ascontiguousarray(x[b].T),
                "Wall": np.ascontiguousarray(
                    np.concatenate([Wq[:, qs], Wk[:, ks], Wv[:, ks]], axis=1)
                ),
                "bqkv": np.ascontiguousarray(
                    np.concatenate([bq[qs], bk[ks], bv[ks]])[:, None]
                ),
                "Wo": np.ascontiguousarray(Wo[qs, :]),
                "bo4": np.ascontiguousarray(bo / KV),
            }
        )
    res = run_bass_kernel_spmd(_program(), in_maps, list(range(N_CORES)))
    parts = [res.results[i]["out"] for i in range(N_CORES)]
    return np.stack(
        [parts[b * KV] + parts[b * KV + 1] + parts[b * KV + 2] + parts[b * KV + 3]
         for b in range(B)]
    )


# revision 11
# speedup vs baseline: 1.0450x; 1.0450x over previous
"""Causal GQA self-attention block (B=2, S=2048, HID=768, 12 q heads, 4 kv heads)
distributed over 8 Trainium2 NeuronCores.

Sharding: one core per (batch, kv-head-group) pair -> 8 perfectly balanced
shards.  Each core projects q/k/v for its group (3 query heads + 1 kv head),
runs causal attention for its 3 heads over the full sequence, and applies its
row-shard of the output projection, producing a partial [S, HID] output.  The
host sums the 4 partials per batch (the row-sharded matmul unshard step) and
stacks the 2 batches.

On-device design (per core):
  xT [768, 2048] hidden-major activations (host passes x[b].T); q/k/v are
  produced feature-major ([64, S]), so scores come out transposed:
  st[sk, sq] = kT_tile.T @ qT.  Softmax over sk (the partition axis) is
  realized without any partition reduction: an all-ones row appended to V
  makes the denominators fall out of the attention @ V' matmul, and a K=1
  broadcast matmul spreads 1/denom back over the 64 output partitions.
  Causal masking is done on the PE: an eye(128)-weighted matmul accumulates a
  -1e30 staircase tile into the score PSUM of diagonal blocks before exp, so
  masked entries exp to exactly 0.  Fully-masked leading columns of diagonal
  blocks are skipped outright (clamped so the moving dim stays >= 256).
All matmuls run in float32r (full PE rate for moving dim >= 256).
"""

import numpy as np

import concourse.bass as bass
import concourse.mybir as mybir
import concourse.tile as tile
from concourse.bass_utils import run_bass_kernel_spmd

F32 = mybir.dt.float32
F32R = mybir.dt.float32r
AF = mybir.ActivationFunctionType

B, S, HID = 2, 2048, 768
KV, G, D = 4, 3, 64
SCALE = D**-0.5
N_CORES = 8

ST = S // 128  # 16 sequence tiles of 128
SC = S // 512  # 4 sequence chunks of 512
HC = HID // 128  # 6 hidden chunks of 128
NG = G + 2  # projection feature groups: q0 q1 q2 k v
NEG = -1.0e30


def _split_excess_waits(nc, max_waits=1):
    """This walrus build rejects >1 sync wait per instruction; move extras
    onto engine-matched NoOps placed right before the instruction."""
    for f in nc.m.functions:
        for bb in f.blocks:
            insts = list(bb.instructions)
            out = []
            changed = False
            for inst in insts:
                si = getattr(inst, "sync_info", None)
                waits = list(si.on_wait) if si is not None else []
                if len(waits) > max_waits:
                    for w in waits[max_waits:]:
                        out.append(
                            mybir.InstNoOp(
                                name=nc.get_next_instruction_name(),
                                sync_info=mybir.SyncInfo(on_wait=[w], on_update=[]),
                                bass_nofuse=True,
                                engine=inst.engine,
                            )
                        )
                    si.on_wait = waits[:max_waits]
                    changed = True
                out.append(inst)
            if changed:
                try:
                    bb.instructions = out
                except Exception:
                    cur = bb.instructions
                    cur.clear()
                    cur.extend(out)


def _neg_masks():
    # Diagonal score tile p (sk-tile p within an sq chunk): entry (i, q) is
    # masked iff 128*p + i > q; masked entries get -1e30 added pre-exp.
    m = np.zeros((128, 4, 512), np.float32)
    q = np.arange(512)[None, :]
    i = np.arange(128)[:, None]
    for p in range(4):
        m[:, p, :] = np.where(128 * p + i > q, NEG, 0.0).astype(np.float32)
    return m


def _build_program():
    nc = bass.Bass()
    xT = nc.dram_tensor("xT", [HID, S], F32, kind="ExternalInput")
    Wall = nc.dram_tensor("Wall", [HID, NG * D], F32, kind="ExternalInput")
    bqkv = nc.dram_tensor("bqkv", [NG * D, 1], F32, kind="ExternalInput")
    Wo = nc.dram_tensor("Wo", [G * D, HID], F32, kind="ExternalInput")
    bo4 = nc.dram_tensor("bo4", [HID], F32, kind="ExternalInput")
    out = nc.dram_tensor("out", [S, HID], F32, kind="ExternalOutput")

    nmask_d = nc.inline_tensor(_neg_masks(), name="nmasks")
    eye64_d = nc.inline_tensor(np.eye(D, dtype=np.float32), name="eye64")
    eye128_d = nc.inline_tensor(np.eye(128, dtype=np.float32), name="eye128")
    ones_d = nc.inline_tensor(np.ones((1, D), np.float32), name="ones64")
    onescol_d = nc.inline_tensor(np.ones((128, ST, 1), np.float32), name="onescol")

    bo4_bcast = bass.AP(tensor=bo4.ap().tensor, offset=0, ap=[[0, 128], [1, HID]])

    with tile.TileContext(nc) as tc:
        with (
            tc.tile_pool(name="persist", bufs=1) as pp,
            tc.tile_pool(name="exp", bufs=6) as ep,
            tc.tile_pool(name="stage", bufs=3) as sp,
            tc.tile_pool(name="ps_mm", bufs=2, space="PSUM") as ps_mm,
            tc.tile_pool(name="ps_acc", bufs=2, space="PSUM") as ps_acc,
            tc.tile_pool(name="ps_out", bufs=2, space="PSUM") as ps_out,
        ):
            # ---- PE warmup: keep HAM busy while input DMAs stream ---------------
            wz = pp.tile([128, 512], F32R, tag="warmz")
            nc.vector.memset(wz.bitcast(F32), 0.0)
            wps = ps_mm.tile([128, 512], F32, tag="mm")
            for w in range(18):
                nc.tensor.matmul(wps, wz[:, 0:128], wz, start=(w == 0), stop=(w == 17))
            # consume the warmup psum so it is not dead-code eliminated
            warm_sink = pp.tile([1, 1], F32, tag="warmsink")
            nc.vector.tensor_copy(warm_sink, wps[0:1, 0:1])

            # ---- resident inputs -------------------------------------------------
            xt = pp.tile([128, HC, S], F32R, tag="xT")
            for c in range(HC):
                nc.sync.dma_start(
                    out=xt[:, c, :], in_=xT[c * 128 : (c + 1) * 128, :].bitcast(F32R)
                )
            wt = pp.tile([128, HC, NG * D], F32R, tag="Wall")
            for c in range(HC):
                nc.sync.dma_start(
                    out=wt[:, c, :],
                    in_=Wall[c * 128 : (c + 1) * 128, :].bitcast(F32R),
                )
            bt = pp.tile([D, NG], F32, tag="bqkv")
            for g in range(NG):
                nc.sync.dma_start(out=bt[:, g : g + 1], in_=bqkv[g * D : (g + 1) * D, :])
            wo = pp.tile([D, G, HID], F32R, tag="Wo")
            for h in range(G):
                nc.sync.dma_start(
                    out=wo[:, h, :], in_=Wo[h * D : (h + 1) * D, :].bitcast(F32R)
                )
            bo_t = pp.tile([128, HID], F32, tag="bo4")
            nc.sync.dma_start(out=bo_t, in_=bo4_bcast)
            nm = pp.tile([128, 4, 512], F32R, tag="nmasks")
            nc.sync.dma_start(out=nm, in_=nmask_d.ap().bitcast(F32R))
            eye64 = pp.tile([D, D], F32R, tag="eye64")
            nc.sync.dma_start(out=eye64, in_=eye64_d.ap().bitcast(F32R))
            eye128 = pp.tile([128, 128], F32R, tag="eye128")
            nc.sync.dma_start(out=eye128, in_=eye128_d.ap().bitcast(F32R))
            # all-ones row living at partition D (=64) so the K=1 broadcast
            # matmul's operands share a base partition
            ones_t = pp.tile([D + 1, D], F32R, tag="ones")
            nc.sync.dma_start(out=ones_t[D : D + 1, :], in_=ones_d.ap().bitcast(F32R))

            # ---- q/k/v projections (feature-major) -------------------------------
            # gT[g] [64, S] = (x @ Wall[:, g*64:(g+1)*64]).T + b[g]
            gts = [
                pp.tile([D, S], F32R, tag=f"gT{g}", name=f"gT{g}") for g in range(NG)
            ]
            for g in range(NG):
                for j in range(SC):
                    acc = ps_acc.tile([D, 512], F32, tag="acc")
                    for c in range(HC):
                        nc.tensor.matmul(
                            acc,
                            wt[:, c, g * D : (g + 1) * D],
                            xt[:, c, j * 512 : (j + 1) * 512],
                            start=(c == 0),
                            stop=(c == HC - 1),
                        )
                    nc.vector.tensor_scalar_add(
                        gts[g][:, j * 512 : (j + 1) * 512], acc, bt[:, g : g + 1]
                    )
            qts, kt, vt = gts[:G], gts[G], gts[G + 1]

            # ---- v' = [v | 1] in sequence-major layout ---------------------------
            vv = pp.tile([128, ST, D + 1], F32R, tag="vv")
            for t in range(ST):
                tp = ps_acc.tile([128, D], F32R, tag="acc")
                nc.tensor.transpose(tp, vt[:, t * 128 : (t + 1) * 128], eye64)
                nc.vector.tensor_copy(vv[:, t, 0:D], tp)
            nc.sync.dma_start(out=vv[:, :, D : D + 1], in_=onescol_d.ap().bitcast(F32R))

            # ---- attention (unnormalized) per head -------------------------------
            ots = [
                pp.tile([D + 1, S], F32R, tag=f"oT{h}", name=f"oT{h}") for h in range(G)
            ]
            for h in range(G):
                qt = qts[h]
                for j in range(SC):
                    n_i = 4 * (j + 1)
                    oacc = ps_acc.tile([D + 1, 512], F32, tag="acc")
                    pend = None  # software-pipeline the AV matmul one step back
                    for i in range(n_i):
                        p = i - 4 * j  # diagonal position (>=0 on causal edge)
                        lo = min(128 * p, 256) if p >= 0 else 0
                        st = ps_mm.tile([128, 512], F32, tag="mm")
                        nc.tensor.matmul(
                            st[:, lo:512],
                            kt[:, i * 128 : (i + 1) * 128],
                            qt[:, j * 512 + lo : (j + 1) * 512],
                            start=True,
                            stop=(p < 0),
                        )
                        if p >= 0:
                            nc.tensor.matmul(
                                st[:, lo:512],
                                eye128,
                                nm[:, p, lo:512],
                                start=False,
                                stop=True,
                            )
                        e = ep.tile([128, 512], F32R, tag="e")
                        nc.scalar.activation(
                            out=e[:, lo:512],
                            in_=st[:, lo:512],
                            func=AF.Exp,
                            scale=SCALE,
                        )
                        if pend is not None:
                            nc.tensor.matmul(
                                oacc[:, pend[2] : 512],
                                vv[:, pend[1], :],
                                pend[0][:, pend[2] : 512],
                                start=(pend[1] == 0),
                                stop=False,
                            )
                        pend = (e, i, lo)
                    nc.tensor.matmul(
                        oacc[:, pend[2] : 512],
                        vv[:, pend[1], :],
                        pend[0][:, pend[2] : 512],
                        start=(pend[1] == 0),
                        stop=True,
                    )
                    nc.scalar.activation(
                        out=ots[h][:, j * 512 : (j + 1) * 512], in_=oacc, func=AF.Copy
                    )
                # denominators -> reciprocals (overlaps the next head's compute)
                with nc.allow_low_precision(reason="f32r denominators"):
                    nc.vector.reciprocal(
                        out=ots[h][D : D + 1, :], in_=ots[h][D : D + 1, :]
                    )

            # ---- normalize + output projection, chunk by chunk -------------------
            for j in range(SC):
                for h in range(G):
                    bc = ps_acc.tile([D, 512], F32, tag="acc")
                    nc.tensor.matmul(
                        bc,
                        ones_t[D : D + 1, :],
                        ots[h][D : D + 1, j * 512 : (j + 1) * 512],
                        start=True,
                        stop=True,
                    )
                    nc.vector.tensor_mul(
                        ots[h][0:D, j * 512 : (j + 1) * 512],
                        ots[h][0:D, j * 512 : (j + 1) * 512],
                        bc.bitcast(F32R),
                    )
                for t in range(4 * j, 4 * (j + 1)):
                    po = ps_out.tile([128, HID], F32, tag="out")
                    for h in range(G):
                        lhs = ots[h][0:D, t * 128 : (t + 1) * 128]
                        nc.tensor.matmul(
                            po[:, 0:512],
                            lhs,
                            wo[:, h, 0:512],
                            start=(h == 0),
                            stop=(h == G - 1),
                        )
                        nc.tensor.matmul(
                            po[:, 512:HID],
                            lhs,
                            wo[:, h, 512:HID],
                            start=(h == 0),
                            stop=(h == G - 1),
                        )
                    stg = sp.tile([128, HID], F32, tag="stg")
                    nc.vector.tensor_add(stg, po, bo_t)
                    nc.sync.dma_start(out=out[t * 128 : (t + 1) * 128, :], in_=stg)

    _split_excess_waits(nc)
    return nc


_PROGRAM = None


def _program():
    global _PROGRAM
    if _PROGRAM is None:
        _PROGRAM = _build_program()
    return _PROGRAM


def kernel(x, Wq, bq, Wk, bk, Wv, bv, Wo, bo, **_):
    x = np.asarray(x, np.float32)
    in_maps = []
    for core in range(N_CORES):
        b, kv = divmod(core, KV)
        qs = slice(kv * G * D, (kv + 1) * G * D)
        ks = slice(kv * D, (kv + 1) * D)
        in_maps.append(
            {
                "xT": np.ascontiguousarray(x[b].T),
                "Wall": np.ascontiguousarray(
                    np.concatenate([Wq[:, qs], Wk[:, ks], Wv[:, ks]], axis=1)
                ),
                "bqkv": np.ascontiguousarray(
                    np.concatenate([bq[qs], bk[ks], bv[ks]])[:, None]
                ),
                "Wo": np.ascontiguousarray(Wo[qs, :]),
                "bo4": np.ascontiguousarray(bo / KV),
            }
        )
    res = run_bass_kernel_spmd(_program(), in_maps, list(range(N_CORES)))
    parts = [res.results[i]["out"] for i in range(N_CORES)]
    return np.stack(
        [
            parts[b * KV] + parts[b * KV + 1] + parts[b * KV + 2] + parts[b * KV + 3]
            for b in range(B)
        ]
    )
